# revision 7
# baseline (speedup 1.0000x reference)
"""Trainium2 Bass kernel for nn_HardAttention (L == S branch).

Math (from the reference, with L == S so the one-hot gather is identity):
    mix      = context                                    # [B, L, D]
    combined = concat(mix, output, axis=2)                # [B, L, 2D]
    out      = tanh(combined @ W.T + b)                   # [B, L, D]
    attn     = broadcast identity one-hot                 # [B, L, S], constant

Strategy: pure data parallel over the 8 NeuronCores — flatten B*L = 16384
tokens, 2048 tokens per core. On each core compute a [2048, 1024] x
[1024, 512] GEMM with fused tanh(+bias).

Device layout: the contraction dim (d = 1024) must live on SBUF partitions
for the PE, so the host pre-transposes the activations to x^T [1024, T] and
the weight to W^T, and the kernel computes y^T [512 out-chan, T tokens]
(W chunks stationary, tokens streaming). The host transposes y^T back.
Activations/weights are fed in fp16 (fp32 PSUM accumulation): full-rate PE,
half the input DMA bytes, FWL fast weight loads.

Pipelining: tokens are processed in 4 chunks of 512; each chunk is one
contiguous 1 MB DMA carrying all 8 k-slices, so the PE starts after the
first MB and PSUM groups double-buffer across chunks. Weights travel on
the ACT HWDGE ring concurrently with x on the SP ring; outputs also go on
the ACT ring. Junk matmuls warm the PE clock gate during the lead-in.
"""

import numpy as np

B, L, S, D = 4, 4096, 4096, 512
CORES = 8
T = (B * L) // CORES      # 2048 tokens per core
K = 2 * D                 # 1024 contraction dim
P = 128                   # partitions
KT = K // P               # 8 contraction tiles
OC = D // P               # 4 output-channel tiles
TCH = 512                 # tokens per chunk (= matmul moving dim)
NCH = T // TCH            # 4 chunks per core
WARM = 10                 # PE warm-up matmuls

_COMPILED = None


def _build():
    import concourse.bacc as bacc
    import concourse.mybir as mybir
    from concourse import bass
    from concourse.tile import TileContext

    f32 = mybir.dt.float32
    f16 = mybir.dt.float16

    nc = bacc.Bacc(
        "TRN2",
        target_bir_lowering=False,
        debug=False,
        enable_asserts=False,
        num_devices=CORES,
    )

    xt = nc.declare_dram_parameter("xt", [P, NCH * KT * TCH], f16, isOutput=False)
    wt = nc.declare_dram_parameter("wt", [P, KT * D], f16, isOutput=False)
    bc = nc.declare_dram_parameter("bc", [P, OC], f32, isOutput=False)
    yt = nc.declare_dram_parameter("yt", [D, T], f32, isOutput=True)

    with TileContext(nc) as tc:
        with (
            tc.tile_pool(name="const", bufs=1) as cp,
            tc.tile_pool(name="xp", bufs=NCH) as xp,
            tc.tile_pool(name="yp", bufs=2) as yp,
            tc.tile_pool(name="pp", bufs=8, space=bass.MemorySpace.PSUM) as pp,
        ):
            # weights + bias via SWDGE (gpsimd) so the ACT engine queue
            # holds nothing but activations
            b_t = cp.tile([P, OC], f32, tag="bias", name="b_t")
            nc.gpsimd.dma_start(b_t[:], bc[:])
            w_t = cp.tile([P, KT * D], f16, tag="w", name="w_t")
            nc.gpsimd.dma_start(w_t[:], wt[:])

            # x: one contiguous 1 MB DMA per chunk (all 8 k-slices),
            # separate tiles so deps are chunk-granular
            cstep = KT * TCH
            x_ts = []
            for ch in range(NCH):
                x_c = xp.tile([P, cstep], f16, tag="x", name=f"x_{ch}")
                nc.sync.dma_start(x_c[:], xt[:, ch * cstep : (ch + 1) * cstep])
                x_ts.append(x_c)

            # PE warm-up: junk matmuls lift the HAM clock gate during
            # lead-in (junk reads uninitialized SBUF on purpose — results
            # are discarded by the start=True of the first real matmul)
            junk = cp.tile([P, TCH], f16, tag="junk", name="junk")
            nc.vector.memset(junk[:], 0.0)
            ps_w = pp.tile([P, TCH], f32, tag="ps", name="ps_warm")
            for _ in range(WARM):
                nc.tensor.matmul(
                    ps_w[:], junk[:, :P], junk[:], start=True, stop=True
                )

            y_ts = {}
            for ch in range(NCH):
                ps = [pp.tile([P, TCH], f32, tag="ps", name=f"ps_{ch}_{oc}")
                      for oc in range(OC)]
                for k in range(KT):
                    rhs = x_ts[ch][:, k * TCH : (k + 1) * TCH]
                    for oc in range(OC):
                        nc.tensor.matmul(
                            ps[oc][:],
                            w_t[:, k * D + oc * P : k * D + (oc + 1) * P],
                            rhs,
                            start=(k == 0),
                            stop=(k == KT - 1),
                        )

                pair, half = divmod(ch, 2)
                if half == 0:
                    y_ts[pair] = yp.tile(
                        [P, OC * 2 * TCH], f32, tag="y", name=f"y_{pair}"
                    )
                y_t = y_ts[pair]
                for oc in range(OC):
                    nc.scalar.activation(
                        y_t[:, oc * 2 * TCH + half * TCH :
                               oc * 2 * TCH + (half + 1) * TCH],
                        ps[oc][:],
                        mybir.ActivationFunctionType.Tanh,
                        bias=b_t[:, oc : oc + 1],
                    )
                if half == 1:
                    # one 2 MB DMA for the whole pair: SBUF [p, oc, t] ->
                    # DRAM yt[(oc p), t] scatter via 3D AP
                    out_ap = yt[:].rearrange("(oc p) t -> p oc t", p=P)[
                        :, :, pair * 2 * TCH : (pair + 1) * 2 * TCH
                    ]
                    in_ap = y_t[:].rearrange("p (oc t) -> p oc t", oc=OC)
                    nc.sync.dma_start(out_ap, in_ap)

    nc.compile()
    return nc


def _get_compiled():
    global _COMPILED
    if _COMPILED is None:
        _COMPILED = _build()
    return _COMPILED


def _prep_inputs(output, context, W, b):
    out_f = np.asarray(output, dtype=np.float32).reshape(B * L, D)
    ctx_f = np.asarray(context, dtype=np.float32).reshape(B * L, D)
    comb = np.concatenate([ctx_f, out_f], axis=1)          # [16384, 1024]
    xt_all = np.ascontiguousarray(comb.T.astype(np.float16))  # [1024, 16384]

    # wt[p, k*D + o] = W[o, k*P + p]
    w_host = np.ascontiguousarray(
        np.asarray(W, dtype=np.float32).T.reshape(KT, P, D).transpose(1, 0, 2)
    ).reshape(P, KT * D).astype(np.float16)
    bc_host = np.ascontiguousarray(
        np.asarray(b, dtype=np.float32).reshape(OC, P).T
    )

    in_maps = []
    for c in range(CORES):
        xt_c = xt_all[:, c * T : (c + 1) * T]              # [1024, 2048]
        # -> [P, NCH*KT*TCH]: chunk-major, then k, then token
        xt_c = np.ascontiguousarray(
            xt_c.reshape(KT, P, NCH, TCH).transpose(1, 2, 0, 3)
        ).reshape(P, NCH * KT * TCH)
        in_maps.append({"xt": xt_c, "wt": w_host, "bc": bc_host})
    return in_maps


def _run(output, context, W, b, trace=False, trace_cores=None, tmpdir=None):
    from concourse.bass_utils import run_bass_kernel_spmd

    nc = _get_compiled()
    in_maps = _prep_inputs(output, context, W, b)
    res = run_bass_kernel_spmd(
        nc,
        in_maps,
        list(range(CORES)),
        trace=trace,
        trace_cores=trace_cores,
        tmpdir=tmpdir,
    )
    y = np.empty((B * L, D), dtype=np.float32)
    for c in range(CORES):
        y[c * T : (c + 1) * T] = res.results[c]["yt"].T
    return y.reshape(B, L, D), res


def kernel(output, context, W, b, di=None):
    y, _ = _run(output, context, W, b)
    attn = np.broadcast_to(
        np.eye(L, S, dtype=np.float32)[None], (B, L, S)
    )
    return y, attn


# revision 8
# speedup vs baseline: 1.0683x; 1.0683x over previous
"""Trainium2 Bass kernel for nn_HardAttention (L == S branch).

Math (from the reference, with L == S so the one-hot gather is identity):
    mix      = context                                    # [B, L, D]
    combined = concat(mix, output, axis=2)                # [B, L, 2D]
    out      = tanh(combined @ W.T + b)                   # [B, L, D]
    attn     = broadcast identity one-hot                 # [B, L, S], constant

Strategy: pure data parallel over the 8 NeuronCores — flatten B*L = 16384
tokens, 2048 tokens per core. On each core compute a [2048, 1024] x
[1024, 512] GEMM with fused tanh(+bias).

Device layout: the contraction dim (d = 1024) must live on SBUF partitions
for the PE, so the host pre-transposes the activations to x^T [1024, T] and
the weight to W^T, and the kernel computes y^T [512 out-chan, T tokens]
(W chunks stationary, tokens streaming). The host transposes y^T back.
Activations/weights are fed in fp16 (fp32 PSUM accumulation): full-rate PE,
half the input DMA bytes, FWL fast weight loads.

Pipelining: tokens are processed in 4 chunks of 512; each chunk is one
contiguous 1 MB DMA carrying all 8 k-slices, so the PE starts after the
first MB and PSUM groups double-buffer across chunks. Weights travel on
the ACT HWDGE ring concurrently with x on the SP ring; outputs also go on
the ACT ring. Junk matmuls warm the PE clock gate during the lead-in.
"""

import numpy as np

B, L, S, D = 4, 4096, 4096, 512
CORES = 8
T = (B * L) // CORES      # 2048 tokens per core
K = 2 * D                 # 1024 contraction dim
P = 128                   # partitions
KT = K // P               # 8 contraction tiles
OC = D // P               # 4 output-channel tiles
TCH = 512                 # tokens per chunk (= matmul moving dim)
NCH = T // TCH            # 4 chunks per core
WARM = 40                 # PE warm-up matmuls (tiny)

_COMPILED = None


def _build():
    import concourse.bacc as bacc
    import concourse.mybir as mybir
    from concourse import bass
    from concourse.tile import TileContext

    f32 = mybir.dt.float32
    f16 = mybir.dt.float16

    nc = bacc.Bacc(
        "TRN2",
        target_bir_lowering=False,
        debug=False,
        enable_asserts=False,
        num_devices=CORES,
    )

    xt = nc.declare_dram_parameter("xt", [P, NCH * KT * TCH], f16, isOutput=False)
    wt = nc.declare_dram_parameter("wt", [P, KT * D], f16, isOutput=False)
    bc = nc.declare_dram_parameter("bc", [P, OC], f32, isOutput=False)
    yt = nc.declare_dram_parameter("yt", [D, T], f32, isOutput=True)

    with TileContext(nc) as tc:
        with (
            tc.tile_pool(name="const", bufs=1) as cp,
            tc.tile_pool(name="xp", bufs=NCH) as xp,
            tc.tile_pool(name="yp", bufs=8) as yp,
            tc.tile_pool(name="pp", bufs=8, space=bass.MemorySpace.PSUM) as pp,
        ):
            # All input DMAs on the SP (sync) HWDGE ring: FIFO per ring
            # guarantees completion order bias -> w_a -> x0 -> w_b -> x1..x3.
            # The ACT engine queue holds nothing but activations.
            cstep = KT * TCH
            b_t = cp.tile([P, OC], f32, tag="bias", name="b_t")
            nc.sync.dma_start(b_t[:], bc[:])
            w_t = cp.tile([P, KT * D], f16, tag="w", name="w_t")
            nc.sync.dma_start(w_t[:, : KT * D // 2], wt[:, : KT * D // 2])

            x_ts = []
            x0 = xp.tile([P, cstep], f16, tag="x", name="x_0")
            nc.sync.dma_start(x0[:], xt[:, :cstep])
            x_ts.append(x0)
            nc.sync.dma_start(w_t[:, KT * D // 2 :], wt[:, KT * D // 2 :])
            for ch in range(1, NCH):
                x_c = xp.tile([P, cstep], f16, tag="x", name=f"x_{ch}")
                nc.sync.dma_start(x_c[:], xt[:, ch * cstep : (ch + 1) * cstep])
                x_ts.append(x_c)

            # PE warm-up: tiny junk matmuls off the bias tile (first DMA to
            # land) lift the HAM clock gate during the DMA lead-in.
            ps_w = pp.tile([P, TCH], f32, tag="ps", name="ps_warm")
            for _ in range(WARM):
                nc.tensor.matmul(
                    ps_w[:OC, :OC], b_t[:], b_t[:], start=True, stop=True
                )

            y_ts = {}
            for ch in range(NCH):
                ps = [pp.tile([P, TCH], f32, tag="ps", name=f"ps_{ch}_{oc}")
                      for oc in range(OC)]
                for k in range(KT):
                    rhs = x_ts[ch][:, k * TCH : (k + 1) * TCH]
                    for oc in range(OC):
                        nc.tensor.matmul(
                            ps[oc][:],
                            w_t[:, k * D + oc * P : k * D + (oc + 1) * P],
                            rhs,
                            start=(k == 0),
                            stop=(k == KT - 1),
                        )

                pair, half = divmod(ch, 2)
                for oc in range(OC):
                    if half == 0:
                        y_ts[(pair, oc)] = yp.tile(
                            [P, 2 * TCH], f32, tag="y", name=f"y_{pair}_{oc}"
                        )
                    y_t = y_ts[(pair, oc)]
                    nc.scalar.activation(
                        y_t[:, half * TCH : (half + 1) * TCH],
                        ps[oc][:],
                        mybir.ActivationFunctionType.Tanh,
                        bias=b_t[:, oc : oc + 1],
                    )
                    if half == 1:
                        nc.sync.dma_start(
                            yt[oc * P : (oc + 1) * P,
                               pair * 2 * TCH : (pair + 1) * 2 * TCH],
                            y_t[:],
                        )

    nc.compile()
    return nc


def _get_compiled():
    global _COMPILED
    if _COMPILED is None:
        _COMPILED = _build()
    return _COMPILED


def _prep_inputs(output, context, W, b):
    out_f = np.asarray(output, dtype=np.float32).reshape(B * L, D)
    ctx_f = np.asarray(context, dtype=np.float32).reshape(B * L, D)
    comb = np.concatenate([ctx_f, out_f], axis=1)          # [16384, 1024]
    xt_all = np.ascontiguousarray(comb.T.astype(np.float16))  # [1024, 16384]

    # wt[p, k*D + o] = W[o, k*P + p]
    w_host = np.ascontiguousarray(
        np.asarray(W, dtype=np.float32).T.reshape(KT, P, D).transpose(1, 0, 2)
    ).reshape(P, KT * D).astype(np.float16)
    bc_host = np.ascontiguousarray(
        np.asarray(b, dtype=np.float32).reshape(OC, P).T
    )

    in_maps = []
    for c in range(CORES):
        xt_c = xt_all[:, c * T : (c + 1) * T]              # [1024, 2048]
        # -> [P, NCH*KT*TCH]: chunk-major, then k, then token
        xt_c = np.ascontiguousarray(
            xt_c.reshape(KT, P, NCH, TCH).transpose(1, 2, 0, 3)
        ).reshape(P, NCH * KT * TCH)
        in_maps.append({"xt": xt_c, "wt": w_host, "bc": bc_host})
    return in_maps


def _run(output, context, W, b, trace=False, trace_cores=None, tmpdir=None):
    from concourse.bass_utils import run_bass_kernel_spmd

    nc = _get_compiled()
    in_maps = _prep_inputs(output, context, W, b)
    res = run_bass_kernel_spmd(
        nc,
        in_maps,
        list(range(CORES)),
        trace=trace,
        trace_cores=trace_cores,
        tmpdir=tmpdir,
    )
    y = np.empty((B * L, D), dtype=np.float32)
    for c in range(CORES):
        y[c * T : (c + 1) * T] = res.results[c]["yt"].T
    return y.reshape(B, L, D), res


def kernel(output, context, W, b, di=None):
    y, _ = _run(output, context, W, b)
    attn = np.broadcast_to(
        np.eye(L, S, dtype=np.float32)[None], (B, L, S)
    )
    return y, attn


# revision 9
# speedup vs baseline: 1.1188x; 1.0473x over previous
"""Trainium2 Bass kernel for nn_HardAttention (L == S branch).

Math (from the reference, with L == S so the one-hot gather is identity):
    mix      = context                                    # [B, L, D]
    combined = concat(mix, output, axis=2)                # [B, L, 2D]
    out      = tanh(combined @ W.T + b)                   # [B, L, D]
    attn     = broadcast identity one-hot                 # [B, L, S], constant

Strategy: pure data parallel over the 8 NeuronCores — flatten B*L = 16384
tokens, 2048 tokens per core. On each core compute a [2048, 1024] x
[1024, 512] GEMM with fused tanh(+bias).

Device layout: the contraction dim (d = 1024) must live on SBUF partitions
for the PE, so the host pre-transposes the activations to x^T [1024, T] and
the weight to W^T, and the kernel computes y^T [512 out-chan, T tokens]
(W chunks stationary, tokens streaming). The host transposes y^T back.
Activations/weights are fed in fp16 (fp32 PSUM accumulation): full-rate PE,
half the input DMA bytes, FWL fast weight loads.

Pipelining: tokens are processed in 4 chunks of 512; each chunk is one
contiguous 1 MB DMA carrying all 8 k-slices, so the PE starts after the
first MB and PSUM groups double-buffer across chunks. Weights travel on
the ACT HWDGE ring concurrently with x on the SP ring; outputs also go on
the ACT ring. Junk matmuls warm the PE clock gate during the lead-in.
"""

import numpy as np

B, L, S, D = 4, 4096, 4096, 512
CORES = 8
T = (B * L) // CORES      # 2048 tokens per core
K = 2 * D                 # 1024 contraction dim
P = 128                   # partitions
KT = K // P               # 8 contraction tiles
OC = D // P               # 4 output-channel tiles
TCH = 512                 # tokens per chunk (= matmul moving dim)
NCH = T // TCH            # 4 chunks per core
WARM = 8                  # PE warm-up matmuls

_COMPILED = None


def _build():
    import concourse.bacc as bacc
    import concourse.mybir as mybir
    from concourse import bass
    from concourse.tile import TileContext

    f32 = mybir.dt.float32
    f16 = mybir.dt.float16

    nc = bacc.Bacc(
        "TRN2",
        target_bir_lowering=False,
        debug=False,
        enable_asserts=False,
        num_devices=CORES,
    )

    xt = nc.declare_dram_parameter("xt", [P, NCH * KT * TCH], f16, isOutput=False)
    wt = nc.declare_dram_parameter("wt", [P, KT * D], f16, isOutput=False)
    bc = nc.declare_dram_parameter("bc", [P, OC], f32, isOutput=False)
    yt = nc.declare_dram_parameter("yt", [D, T], f32, isOutput=True)

    with TileContext(nc) as tc:
        with (
            tc.tile_pool(name="const", bufs=1) as cp,
            tc.tile_pool(name="xp", bufs=NCH) as xp,
            tc.tile_pool(name="yp", bufs=8) as yp,
            tc.tile_pool(name="pp", bufs=8, space=bass.MemorySpace.PSUM) as pp,
        ):
            # All input DMAs on the SP (sync) HWDGE ring: FIFO per ring
            # guarantees completion order bias -> w_a -> x0 -> w_b -> x1..x3.
            # The ACT engine queue holds nothing but activations.
            cstep = KT * TCH
            b_t = cp.tile([P, OC], f32, tag="bias", name="b_t")
            nc.sync.dma_start(b_t[:], bc[:])
            w_t = cp.tile([P, KT * D], f16, tag="w", name="w_t")
            nc.sync.dma_start(w_t[:], wt[:])

            x_ts = []
            for ch in range(NCH):
                x_c = xp.tile([P, cstep], f16, tag="x", name=f"x_{ch}")
                nc.sync.dma_start(x_c[:], xt[:, ch * cstep : (ch + 1) * cstep])
                x_ts.append(x_c)

            # PE warm-up: fp16 junk matmuls lift the HAM clock gate during
            # the DMA lead-in.
            junk = cp.tile([P, TCH], f16, tag="junk", name="junk")
            nc.vector.memset(junk[:], 0.0)
            ps_w = pp.tile([P, TCH], f32, tag="ps", name="ps_warm")
            for _ in range(WARM):
                nc.tensor.matmul(
                    ps_w[:], junk[:, :P], junk[:], start=True, stop=True
                )

            y_ts = {}
            for ch in range(NCH):
                ps = [pp.tile([P, TCH], f32, tag="ps", name=f"ps_{ch}_{oc}")
                      for oc in range(OC)]
                for k in range(KT):
                    rhs = x_ts[ch][:, k * TCH : (k + 1) * TCH]
                    for oc in range(OC):
                        nc.tensor.matmul(
                            ps[oc][:],
                            w_t[:, k * D + oc * P : k * D + (oc + 1) * P],
                            rhs,
                            start=(k == 0),
                            stop=(k == KT - 1),
                        )

                pair, half = divmod(ch, 2)
                for oc in range(OC):
                    if half == 0:
                        y_ts[(pair, oc)] = yp.tile(
                            [P, 2 * TCH], f32, tag="y", name=f"y_{pair}_{oc}"
                        )
                    y_t = y_ts[(pair, oc)]
                    nc.scalar.activation(
                        y_t[:, half * TCH : (half + 1) * TCH],
                        ps[oc][:],
                        mybir.ActivationFunctionType.Tanh,
                        bias=b_t[:, oc : oc + 1],
                    )
                    if half == 1:
                        nc.sync.dma_start(
                            yt[oc * P : (oc + 1) * P,
                               pair * 2 * TCH : (pair + 1) * 2 * TCH],
                            y_t[:],
                        )

    nc.compile()
    return nc


def _get_compiled():
    global _COMPILED
    if _COMPILED is None:
        _COMPILED = _build()
    return _COMPILED


def _prep_inputs(output, context, W, b):
    out_f = np.asarray(output, dtype=np.float32).reshape(B * L, D)
    ctx_f = np.asarray(context, dtype=np.float32).reshape(B * L, D)
    comb = np.concatenate([ctx_f, out_f], axis=1)          # [16384, 1024]
    xt_all = np.ascontiguousarray(comb.T.astype(np.float16))  # [1024, 16384]

    # wt[p, k*D + o] = W[o, k*P + p]
    w_host = np.ascontiguousarray(
        np.asarray(W, dtype=np.float32).T.reshape(KT, P, D).transpose(1, 0, 2)
    ).reshape(P, KT * D).astype(np.float16)
    bc_host = np.ascontiguousarray(
        np.asarray(b, dtype=np.float32).reshape(OC, P).T
    )

    in_maps = []
    for c in range(CORES):
        xt_c = xt_all[:, c * T : (c + 1) * T]              # [1024, 2048]
        # -> [P, NCH*KT*TCH]: chunk-major, then k, then token
        xt_c = np.ascontiguousarray(
            xt_c.reshape(KT, P, NCH, TCH).transpose(1, 2, 0, 3)
        ).reshape(P, NCH * KT * TCH)
        in_maps.append({"xt": xt_c, "wt": w_host, "bc": bc_host})
    return in_maps


def _run(output, context, W, b, trace=False, trace_cores=None, tmpdir=None):
    from concourse.bass_utils import run_bass_kernel_spmd

    nc = _get_compiled()
    in_maps = _prep_inputs(output, context, W, b)
    res = run_bass_kernel_spmd(
        nc,
        in_maps,
        list(range(CORES)),
        trace=trace,
        trace_cores=trace_cores,
        tmpdir=tmpdir,
    )
    y = np.empty((B * L, D), dtype=np.float32)
    for c in range(CORES):
        y[c * T : (c + 1) * T] = res.results[c]["yt"].T
    return y.reshape(B, L, D), res


def kernel(output, context, W, b, di=None):
    y, _ = _run(output, context, W, b)
    attn = np.broadcast_to(
        np.eye(L, S, dtype=np.float32)[None], (B, L, S)
    )
    return y, attn


# revision 11
# speedup vs baseline: 1.1474x; 1.0255x over previous
"""Trainium2 Bass kernel for nn_HardAttention (L == S branch).

Math (from the reference, with L == S so the one-hot gather is identity):
    mix      = context                                    # [B, L, D]
    combined = concat(mix, output, axis=2)                # [B, L, 2D]
    out      = tanh(combined @ W.T + b)                   # [B, L, D]
    attn     = broadcast identity one-hot                 # [B, L, S], constant

Strategy: pure data parallel over the 8 NeuronCores — flatten B*L = 16384
tokens, 2048 tokens per core. On each core compute a [2048, 1024] x
[1024, 512] GEMM with fused tanh(+bias).

Device layout: the contraction dim (d = 1024) must live on SBUF partitions
for the PE, so the host pre-transposes the activations to x^T [1024, T] and
the weight to W^T, and the kernel computes y^T [512 out-chan, T tokens]
(W chunks stationary, tokens streaming). The host transposes y^T back.
Activations/weights are fed in fp16 (fp32 PSUM accumulation): full-rate PE,
half the input DMA bytes, FWL fast weight loads.

Pipelining: tokens are processed in 4 chunks of 512; each chunk is one
contiguous 1 MB DMA carrying all 8 k-slices, so the PE starts after the
first MB and PSUM groups double-buffer across chunks. Weights travel on
the ACT HWDGE ring concurrently with x on the SP ring; outputs also go on
the ACT ring. Junk matmuls warm the PE clock gate during the lead-in.
"""

import numpy as np

B, L, S, D = 4, 4096, 4096, 512
CORES = 8
T = (B * L) // CORES      # 2048 tokens per core
K = 2 * D                 # 1024 contraction dim
P = 128                   # partitions
KT = K // P               # 8 contraction tiles
OC = D // P               # 4 output-channel tiles
TCH = 512                 # tokens per chunk (= matmul moving dim)
NCH = T // TCH            # 4 chunks per core
WARM = 8                  # PE warm-up matmuls

_COMPILED = None


def _build():
    import concourse.bacc as bacc
    import concourse.mybir as mybir
    from concourse import bass
    from concourse.tile import TileContext

    f32 = mybir.dt.float32
    f16 = mybir.dt.float16

    nc = bacc.Bacc(
        "TRN2",
        target_bir_lowering=False,
        debug=False,
        enable_asserts=False,
        num_devices=CORES,
    )

    xt = nc.declare_dram_parameter("xt", [P, NCH * KT * TCH], f16, isOutput=False)
    wt = nc.declare_dram_parameter("wt", [P, KT * D], f16, isOutput=False)
    bc = nc.declare_dram_parameter("bc", [P, OC], f32, isOutput=False)
    yt = nc.declare_dram_parameter("yt", [D, T], f32, isOutput=True)

    with TileContext(nc) as tc:
        with (
            tc.tile_pool(name="const", bufs=1) as cp,
            tc.tile_pool(name="xp", bufs=NCH) as xp,
            tc.tile_pool(name="yp", bufs=8) as yp,
            tc.tile_pool(name="pp", bufs=8, space=bass.MemorySpace.PSUM) as pp,
        ):
            # All input DMAs on the SP (sync) HWDGE ring: FIFO per ring
            # guarantees completion order bias -> w_a -> x0 -> w_b -> x1..x3.
            # The ACT engine queue holds nothing but activations.
            cstep = KT * TCH
            b_t = cp.tile([P, OC], f32, tag="bias", name="b_t")
            nc.sync.dma_start(b_t[:], bc[:])
            # w split per k-slice, interleaved with x chunks on the same
            # FIFO ring: the first matmuls are gated only on w0 + x0.
            w_ts = []
            w0 = cp.tile([P, D], f16, tag="w0", name="w_0")
            nc.sync.dma_start(w0[:], wt[:, :D])
            w_ts.append(w0)

            x_ts = []
            x0 = xp.tile([P, cstep], f16, tag="x", name="x_0")
            nc.sync.dma_start(x0[:], xt[:, :cstep])
            x_ts.append(x0)
            for k in range(1, KT):
                w_k = cp.tile([P, D], f16, tag=f"w{k}", name=f"w_{k}")
                nc.sync.dma_start(w_k[:], wt[:, k * D : (k + 1) * D])
                w_ts.append(w_k)
            for ch in range(1, NCH):
                x_c = xp.tile([P, cstep], f16, tag="x", name=f"x_{ch}")
                nc.sync.dma_start(x_c[:], xt[:, ch * cstep : (ch + 1) * cstep])
                x_ts.append(x_c)

            # PE warm-up: fp16 junk matmuls lift the HAM clock gate during
            # the DMA lead-in.
            junk = cp.tile([P, TCH], f16, tag="junk", name="junk")
            nc.vector.memset(junk[:], 0.0)
            ps_w = pp.tile([P, TCH], f32, tag="ps", name="ps_warm")
            for _ in range(WARM):
                nc.tensor.matmul(
                    ps_w[:], junk[:, :P], junk[:], start=True, stop=True
                )

            for ch in range(NCH):
                ps = [pp.tile([P, TCH], f32, tag="ps", name=f"ps_{ch}_{oc}")
                      for oc in range(OC)]
                for k in range(KT):
                    rhs = x_ts[ch][:, k * TCH : (k + 1) * TCH]
                    for oc in range(OC):
                        nc.tensor.matmul(
                            ps[oc][:],
                            w_ts[k][:, oc * P : (oc + 1) * P],
                            rhs,
                            start=(k == 0),
                            stop=(k == KT - 1),
                        )

                for oc in range(OC):
                    y_t = yp.tile([P, TCH], f32, tag="y", name=f"y_{ch}_{oc}")
                    nc.scalar.activation(
                        y_t[:],
                        ps[oc][:],
                        mybir.ActivationFunctionType.Tanh,
                        bias=b_t[:, oc : oc + 1],
                    )
                    nc.sync.dma_start(
                        yt[oc * P : (oc + 1) * P, ch * TCH : (ch + 1) * TCH],
                        y_t[:],
                    )

    nc.compile()
    return nc


def _get_compiled():
    global _COMPILED
    if _COMPILED is None:
        _COMPILED = _build()
    return _COMPILED


def _prep_inputs(output, context, W, b):
    out_f = np.asarray(output, dtype=np.float32).reshape(B * L, D)
    ctx_f = np.asarray(context, dtype=np.float32).reshape(B * L, D)
    comb = np.concatenate([ctx_f, out_f], axis=1)          # [16384, 1024]
    xt_all = np.ascontiguousarray(comb.T.astype(np.float16))  # [1024, 16384]

    # wt[p, k*D + o] = W[o, k*P + p]
    w_host = np.ascontiguousarray(
        np.asarray(W, dtype=np.float32).T.reshape(KT, P, D).transpose(1, 0, 2)
    ).reshape(P, KT * D).astype(np.float16)
    bc_host = np.ascontiguousarray(
        np.asarray(b, dtype=np.float32).reshape(OC, P).T
    )

    in_maps = []
    for c in range(CORES):
        xt_c = xt_all[:, c * T : (c + 1) * T]              # [1024, 2048]
        # -> [P, NCH*KT*TCH]: chunk-major, then k, then token
        xt_c = np.ascontiguousarray(
            xt_c.reshape(KT, P, NCH, TCH).transpose(1, 2, 0, 3)
        ).reshape(P, NCH * KT * TCH)
        in_maps.append({"xt": xt_c, "wt": w_host, "bc": bc_host})
    return in_maps


def _run(output, context, W, b, trace=False, trace_cores=None, tmpdir=None):
    from concourse.bass_utils import run_bass_kernel_spmd

    nc = _get_compiled()
    in_maps = _prep_inputs(output, context, W, b)
    res = run_bass_kernel_spmd(
        nc,
        in_maps,
        list(range(CORES)),
        trace=trace,
        trace_cores=trace_cores,
        tmpdir=tmpdir,
    )
    y = np.empty((B * L, D), dtype=np.float32)
    for c in range(CORES):
        y[c * T : (c + 1) * T] = res.results[c]["yt"].T
    return y.reshape(B, L, D), res


def kernel(output, context, W, b, di=None):
    y, _ = _run(output, context, W, b)
    attn = np.broadcast_to(
        np.eye(L, S, dtype=np.float32)[None], (B, L, S)
    )
    return y, attn


# revision 12
# speedup vs baseline: 1.2484x; 1.0880x over previous
"""Trainium2 Bass kernel for nn_HardAttention (L == S branch).

Math (from the reference, with L == S so the one-hot gather is identity):
    mix      = context                                    # [B, L, D]
    combined = concat(mix, output, axis=2)                # [B, L, 2D]
    out      = tanh(combined @ W.T + b)                   # [B, L, D]
    attn     = broadcast identity one-hot                 # [B, L, S], constant

Strategy: pure data parallel over the 8 NeuronCores — flatten B*L = 16384
tokens, 2048 tokens per core. On each core compute a [2048, 1024] x
[1024, 512] GEMM with fused tanh(+bias).

Device layout: the contraction dim (d = 1024) must live on SBUF partitions
for the PE, so the host pre-transposes the activations to x^T [1024, T] and
the weight to W^T, and the kernel computes y^T [512 out-chan, T tokens]
(W chunks stationary, tokens streaming). The host transposes y^T back.
Activations/weights are fed in fp16 (fp32 PSUM accumulation): full-rate PE,
half the input DMA bytes, FWL fast weight loads.

Pipelining: tokens are processed in 4 chunks of 512; each chunk is one
contiguous 1 MB DMA carrying all 8 k-slices, so the PE starts after the
first MB and PSUM groups double-buffer across chunks. Weights travel on
the ACT HWDGE ring concurrently with x on the SP ring; outputs also go on
the ACT ring. Junk matmuls warm the PE clock gate during the lead-in.
"""

import numpy as np

B, L, S, D = 4, 4096, 4096, 512
CORES = 8
T = (B * L) // CORES      # 2048 tokens per core
K = 2 * D                 # 1024 contraction dim
P = 128                   # partitions
KT = K // P               # 8 contraction tiles
OC = D // P               # 4 output-channel tiles
TCH = 512                 # tokens per chunk (= matmul moving dim)
NCH = T // TCH            # 4 chunks per core
WARM = 6                  # PE warm-up matmuls

_COMPILED = None


def _build():
    import concourse.bacc as bacc
    import concourse.mybir as mybir
    from concourse import bass
    from concourse.tile import TileContext

    f32 = mybir.dt.float32
    f16 = mybir.dt.float16

    nc = bacc.Bacc(
        "TRN2",
        target_bir_lowering=False,
        debug=False,
        enable_asserts=False,
        num_devices=CORES,
    )

    xt = nc.declare_dram_parameter("xt", [P, NCH * KT * TCH], f16, isOutput=False)
    wt = nc.declare_dram_parameter("wt", [P, KT * D], f16, isOutput=False)
    bc = nc.declare_dram_parameter("bc", [P, OC], f32, isOutput=False)
    yt = nc.declare_dram_parameter("yt", [D, T], f32, isOutput=True)

    with TileContext(nc) as tc:
        with (
            tc.tile_pool(name="const", bufs=1) as cp,
            tc.tile_pool(name="xp", bufs=NCH) as xp,
            tc.tile_pool(name="yp", bufs=8) as yp,
            tc.tile_pool(name="pp", bufs=8, space=bass.MemorySpace.PSUM) as pp,
        ):
            # All input DMAs on the SP (sync) HWDGE ring: FIFO per ring
            # guarantees completion order bias -> w_a -> x0 -> w_b -> x1..x3.
            # The ACT engine queue holds nothing but activations.
            cstep = KT * TCH
            b_t = cp.tile([P, OC], f32, tag="bias", name="b_t")
            nc.sync.dma_start(b_t[:], bc[:])
            # w split per k-slice, interleaved with x chunks on the same
            # FIFO ring: the first matmuls are gated only on w0 + x0.
            w_ts = []
            w0 = cp.tile([P, D], f16, tag="w0", name="w_0")
            nc.sync.dma_start(w0[:], wt[:, :D])
            w_ts.append(w0)

            # chunk 0 split in two half tiles (k 0-3 | k 4-7) so the very
            # first matmuls gate on ~640 KB of leading traffic
            half = cstep // 2
            x0a = xp.tile([P, half], f16, tag="x0a", name="x0a", bufs=1)
            nc.sync.dma_start(x0a[:], xt[:, :half])
            for k in range(1, 4):
                w_k = cp.tile([P, D], f16, tag=f"w{k}", name=f"w_{k}")
                nc.sync.dma_start(w_k[:], wt[:, k * D : (k + 1) * D])
                w_ts.append(w_k)
            x0b = xp.tile([P, half], f16, tag="x0b", name="x0b", bufs=1)
            nc.sync.dma_start(x0b[:], xt[:, half:cstep])
            for k in range(4, KT):
                w_k = cp.tile([P, D], f16, tag=f"w{k}", name=f"w_{k}")
                nc.sync.dma_start(w_k[:], wt[:, k * D : (k + 1) * D])
                w_ts.append(w_k)
            x_ts = [(x0a, x0b)]
            for ch in range(1, NCH):
                x_c = xp.tile([P, cstep], f16, tag="x", name=f"x_{ch}")
                nc.sync.dma_start(x_c[:], xt[:, ch * cstep : (ch + 1) * cstep])
                x_ts.append(x_c)

            # PE warm-up: fp16 junk matmuls lift the HAM clock gate during
            # the DMA lead-in.
            junk = cp.tile([P, TCH], f16, tag="junk", name="junk")
            nc.vector.memset(junk[:], 0.0)
            ps_w = pp.tile([P, TCH], f32, tag="ps", name="ps_warm")
            for _ in range(WARM):
                nc.tensor.matmul(
                    ps_w[:], junk[:, :P], junk[:], start=True, stop=True
                )

            for ch in range(NCH):
                ps = [pp.tile([P, TCH], f32, tag="ps", name=f"ps_{ch}_{oc}")
                      for oc in range(OC)]
                for k in range(KT):
                    if ch == 0:
                        xh = x_ts[0][0] if k < 4 else x_ts[0][1]
                        rhs = xh[:, (k % 4) * TCH : (k % 4 + 1) * TCH]
                    else:
                        rhs = x_ts[ch][:, k * TCH : (k + 1) * TCH]
                    for oc in range(OC):
                        nc.tensor.matmul(
                            ps[oc][:],
                            w_ts[k][:, oc * P : (oc + 1) * P],
                            rhs,
                            start=(k == 0),
                            stop=(k == KT - 1),
                        )

                for oc in range(OC):
                    y_t = yp.tile([P, TCH], f32, tag="y", name=f"y_{ch}_{oc}")
                    nc.scalar.activation(
                        y_t[:],
                        ps[oc][:],
                        mybir.ActivationFunctionType.Tanh,
                        bias=b_t[:, oc : oc + 1],
                    )
                    nc.sync.dma_start(
                        yt[oc * P : (oc + 1) * P, ch * TCH : (ch + 1) * TCH],
                        y_t[:],
                    )

    nc.compile()
    return nc


def _get_compiled():
    global _COMPILED
    if _COMPILED is None:
        _COMPILED = _build()
    return _COMPILED


def _prep_inputs(output, context, W, b):
    out_f = np.asarray(output, dtype=np.float32).reshape(B * L, D)
    ctx_f = np.asarray(context, dtype=np.float32).reshape(B * L, D)
    comb = np.concatenate([ctx_f, out_f], axis=1)          # [16384, 1024]
    xt_all = np.ascontiguousarray(comb.T.astype(np.float16))  # [1024, 16384]

    # wt[p, k*D + o] = W[o, k*P + p]
    w_host = np.ascontiguousarray(
        np.asarray(W, dtype=np.float32).T.reshape(KT, P, D).transpose(1, 0, 2)
    ).reshape(P, KT * D).astype(np.float16)
    bc_host = np.ascontiguousarray(
        np.asarray(b, dtype=np.float32).reshape(OC, P).T
    )

    in_maps = []
    for c in range(CORES):
        xt_c = xt_all[:, c * T : (c + 1) * T]              # [1024, 2048]
        # -> [P, NCH*KT*TCH]: chunk-major, then k, then token
        xt_c = np.ascontiguousarray(
            xt_c.reshape(KT, P, NCH, TCH).transpose(1, 2, 0, 3)
        ).reshape(P, NCH * KT * TCH)
        in_maps.append({"xt": xt_c, "wt": w_host, "bc": bc_host})
    return in_maps


def _run(output, context, W, b, trace=False, trace_cores=None, tmpdir=None):
    from concourse.bass_utils import run_bass_kernel_spmd

    nc = _get_compiled()
    in_maps = _prep_inputs(output, context, W, b)
    res = run_bass_kernel_spmd(
        nc,
        in_maps,
        list(range(CORES)),
        trace=trace,
        trace_cores=trace_cores,
        tmpdir=tmpdir,
    )
    y = np.empty((B * L, D), dtype=np.float32)
    for c in range(CORES):
        y[c * T : (c + 1) * T] = res.results[c]["yt"].T
    return y.reshape(B, L, D), res


def kernel(output, context, W, b, di=None):
    y, _ = _run(output, context, W, b)
    attn = np.broadcast_to(
        np.eye(L, S, dtype=np.float32)[None], (B, L, S)
    )
    return y, attn
